# revision 16
# baseline (speedup 1.0000x reference)
"""Cross-attention Trainium2 kernel (nn_CrossAttention_8486855377137).

Sharding (8 cores): core c = (batch b = c//2, head-group g = c%2).
Each core handles one batch and 4 of the 8 heads (Q/K/V projections
column-sharded by head, wo row-sharded). Full softmax over S on device;
host sums the two partial wo outputs per batch and adds wo bias.

v4: three-engine exp + flipped attnV.
  - Head-major unit-halves: (c2-pair, head, t-width 1024); scores bf16
    64-contract with K stationary per s-tile, Q streamed 1024 cols.
  - exp split across engines: ACT exact exp; DVE/GPSIMD approximate exp
    via Schraudolph affine + f32->i16 round-convert bitcast to bf16
    (error cancels in softmax; sim: +1e-3 at 3/8 offload).
  - attnV flipped: es is the PE stationary (128x128 blocks), V (+ones
    denominator column) streams; out is [t, d|den] so the softmax
    normalize is a per-partition tensor_scalar, then PE transpose back.
  - K path fp8 DoubleRow (ctx fp8 + wk fp8 x512) as before.
"""

import numpy as np
import ml_dtypes

import concourse.bass as bass
import concourse.bacc as bacc
import concourse.tile as tile
import concourse.mybir as mybir
from concourse.bass_utils import run_bass_kernel_spmd

BF16 = mybir.dt.bfloat16
F32 = mybir.dt.float32
FP8 = mybir.dt.float8e4
I16 = mybir.dt.int16
EXP = mybir.ActivationFunctionType.Exp
ADD = mybir.AluOpType.add
MULT = mybir.AluOpType.mult
DR = mybir.MatmulPerfMode.DoubleRow
NPBF16 = ml_dtypes.bfloat16
NPFP8 = ml_dtypes.float8_e4m3

# Problem constants (hardcoded per contract)
B, T, S = 4, 2048, 4096
E, KV = 512, 2048
H, D = 8, 64
GE = 256            # head-group embed width (4 heads x 64)
SCALE = D ** -0.5   # 0.125
WK_SCALE = 512.0    # wk prescale (fp8 denormal avoidance); folded into exp scale
EXP_BIAS = -1.5     # exp(x + bias): cancels in softmax, shrinks es range

N_CORES = 8
P = 128
TW = 1024           # t-width per unit-half
NTP = T // TW       # 2 t-chunks
NSC = S // P        # 32 s-tiles
SGT = 4             # s-tiles per chase production step (512 cols)
NSG = NSC // SGT    # 8 production steps
KV_C = KV // P      # 16 contraction chunks for K/V proj
E_C = E // P        # 4 contraction chunks for Q proj

# Schraudolph constants (bf16: 7 mantissa bits)
SCH_K1 = 128.0 / np.log(2.0)
SCH_CORR = -7.4
SCH_S0 = SCH_K1 * SCALE / WK_SCALE
SCH_S1 = SCH_K1 * EXP_BIAS + 127.0 * 128.0 + SCH_CORR

# exp engine pattern per 8 s-tiles: A=ACT exact, D=DVE schraud.
# (GPSIMD cannot read PSUM, so it can't join the exp stream.)
PATTERN_STEADY = ('A', 'A', 'D', 'A', 'A', 'D', 'A', 'D')
PATTERN_CHASE = ('A',) * 8   # DVE busy with production evictions in chase


def _build_nc():
    nc = bacc.Bacc("TRN2", target_bir_lowering=False, debug=False)

    ctx8 = nc.dram_tensor("ctx8", [KV, S], FP8, kind="ExternalInput")
    xT = nc.dram_tensor("xT", [E, T], BF16, kind="ExternalInput")
    wqT = nc.dram_tensor("wqT", [E, GE], BF16, kind="ExternalInput")
    wkT = nc.dram_tensor("wkT", [KV, GE], FP8, kind="ExternalInput")
    wvT = nc.dram_tensor("wvT", [KV, GE], BF16, kind="ExternalInput")
    woT = nc.dram_tensor("woT", [GE, E], BF16, kind="ExternalInput")
    bq = nc.dram_tensor("bq", [GE], F32, kind="ExternalInput")
    bk = nc.dram_tensor("bk", [GE], F32, kind="ExternalInput")   # x512
    bv = nc.dram_tensor("bv", [GE], F32, kind="ExternalInput")
    ident = nc.dram_tensor("ident", [P, P], BF16, kind="ExternalInput")
    yT = nc.dram_tensor("yT", [E, T], F32, kind="ExternalOutput")

    with tile.TileContext(nc) as tc:
        _kernel_body(tc, nc, ctx8, xT, wqT, wkT, wvT, woT, bq, bk, bv,
                     ident, yT)
    nc.compile()
    return nc


def _kernel_body(tc, nc, ctx8, xT, wqT, wkT, wvT, woT, bq, bk, bv, ident, yT):
    wts = tc.alloc_tile_pool(name="wts", bufs=1)
    persist = tc.alloc_tile_pool(name="persist", bufs=1)

    # ---- DMA kickoff. Big streams on sync queue, weights on gpsimd queue ----
    ctx8_r = ctx8.rearrange("(c p) s -> p c s", p=P)
    yT_r = yT.rearrange("(m p) t -> p m t", p=P)

    wkT_sb = wts.tile([P, KV_C, GE], FP8, tag="wkT")
    nc.gpsimd.dma_start(wkT_sb, wkT.rearrange("(c p) m -> p c m", p=P))
    wqT_sb = wts.tile([P, E_C, GE], BF16, tag="wqT")
    nc.gpsimd.dma_start(wqT_sb, wqT.rearrange("(c p) m -> p c m", p=P))
    bq_sb = wts.tile([P, 2], F32, tag="bq")
    nc.gpsimd.dma_start(bq_sb, bq.rearrange("(c p) -> p c", p=P))
    bk_sb = wts.tile([P, 2], F32, tag="bk")
    nc.gpsimd.dma_start(bk_sb, bk.rearrange("(c p) -> p c", p=P))
    ident_sb = wts.tile([P, P], BF16, tag="ident")
    nc.gpsimd.dma_start(ident_sb, ident.ap())
    woT_sb = wts.tile([P, 2, E], BF16, tag="woT")
    nc.gpsimd.dma_start(woT_sb, woT.rearrange("(c p) m -> p c m", p=P))
    ebias_sb = wts.tile([P, 1], F32, tag="ebias")
    nc.vector.memset(ebias_sb, EXP_BIAS)
    # bv broadcast to all 128 partitions, used along free dim of V
    bv_bc = wts.tile([P, GE], F32, tag="bv_bc")
    bv_ap = bv.ap()
    bv_bcast_src = bass.AP(tensor=bv_ap.tensor, offset=bv_ap.offset,
                           ap=[[0, P]] + list(bv_ap.ap))
    nc.gpsimd.dma_start(out=bv_bc, in_=bv_bcast_src)

    xT_sb = wts.tile([P, E_C, T], BF16, tag="xT")
    wvT_sb = wts.tile([P, KV_C, GE], BF16, tag="wvT")

    # ---- persistent activation tiles ----
    QT_sb = [persist.tile([P, T], BF16, tag=f"QT{c}", name=f"QT{c}") for c in range(2)]
    KT_sb = [persist.tile([P, S], BF16, tag=f"KT{c}", name=f"KT{c}") for c in range(2)]
    # V bf16, head-major: [128, head(4), s-tile(32), 65]; col 64 = 1.0 (den)
    V_sb = persist.tile([P, 4, NSC, 65], BF16, tag="V", name="V")
    nc.vector.memset(V_sb[:, :, :, 64:65], 1.0)
    OcatT = [persist.tile([P, T], BF16, tag=f"Ocat{c}", name=f"Ocat{c}") for c in range(2)]

    with tc.tile_pool(name="aps", bufs=1, space="PSUM") as aps, \
         tc.tile_pool(name="c8pool", bufs=3) as c8pool, \
         tc.tile_pool(name="espool", bufs=10) as espool, \
         tc.tile_pool(name="evpool", bufs=2) as evpool, \
         tc.tile_pool(name="ystg", bufs=2) as ystg:

        ctx8_tiles = {}

        def ctx_dma(sg):
            """Fetch ctx half-group sg (512 cols) in fp8."""
            cols = slice(sg * 512, (sg + 1) * 512)
            t8 = c8pool.tile([P, KV_C, 512], FP8, tag="c8")
            nc.sync.dma_start(t8, ctx8_r[:, :, cols])
            ctx8_tiles[sg] = t8

        def k_group(sg):
            """KT[:, sg cols] for both c2 chunks; fp8 DoubleRow; x512."""
            ctx_t = ctx8_tiles[sg]
            for m in range(2):
                ps = aps.tile([P, 512], F32, tag="kvps", bufs=2,
                              name=f"kps{sg}{m}")
                for cp in range(KV_C // 2):
                    nc.tensor.matmul(
                        ps,
                        wkT_sb[:, 2 * cp:2 * cp + 2, m * P:(m + 1) * P],
                        ctx_t[:, 2 * cp:2 * cp + 2, :],
                        start=(cp == 0), stop=(cp == KV_C // 2 - 1),
                        perf_mode=DR, skip_group_check=True)
                nc.vector.tensor_scalar_add(
                    KT_sb[m][:, sg * 512:(sg + 1) * 512],
                    ps, bk_sb[:, m:m + 1])

        def v_group(sg):
            """V s-tiles of half-group sg; bf16."""
            ctx_t = ctx8_tiles.pop(sg)
            for sp in range(2):
                ps = aps.tile([P, 512], F32, tag="kvps", bufs=2,
                              name=f"vps{sg}{sp}")
                for st4 in (2 * sp, 2 * sp + 1):
                    for c in range(KV_C):
                        nc.tensor.matmul(
                            ps[:, (st4 % 2) * 256:(st4 % 2 + 1) * 256],
                            ctx_t[:, c, st4 * P:(st4 + 1) * P],
                            wvT_sb[:, c, :],
                            start=(c == 0), stop=(c == KV_C - 1),
                            skip_group_check=True)
                for st4 in (2 * sp, 2 * sp + 1):
                    idx = sg * SGT + st4
                    nc.vector.tensor_tensor(
                        V_sb[:, :, idx, 0:64],
                        ps[:, (st4 % 2) * 256:(st4 % 2 + 1) * 256].rearrange(
                            "p (h e) -> p h e", e=64),
                        bv_bc.rearrange("p (h e) -> p h e", e=64),
                        ADD)

        def q_proj(parts):
            for c2, tp in parts:
                for tn in range(2):
                    t = tp * 2 + tn
                    ps = aps.tile([P, 512], F32, tag="kvps", bufs=2,
                                  name=f"qps{c2}{tp}{tn}")
                    for c in range(E_C):
                        nc.tensor.matmul(
                            ps,
                            wqT_sb[:, c, c2 * P:(c2 + 1) * P],
                            xT_sb[:, c, t * 512:(t + 1) * 512],
                            start=(c == 0), stop=(c == E_C - 1),
                            skip_group_check=True)
                    nc.vector.tensor_scalar_add(
                        QT_sb[c2][:, t * 512:(t + 1) * 512], ps,
                        bq_sb[:, c2:c2 + 1])

        def unit_half(c2, h, tp, chase=False, inject=None,
                      pattern=PATTERN_STEADY):
            """Generator: one head, 1024 t-cols, full softmax over S.

            scores: K-tile stationary (64-contract), Q streamed (1024).
            exp: engine per `pattern`. attnV: es stationary 128x128 blocks,
            V+ones streamed -> o_ps [t, 8tb, 65]; normalize per-partition;
            PE-transpose back into OcatT."""
            tcols = slice(tp * TW, (tp + 1) * TW)
            hh = 2 * c2 + h
            prow = slice(h * 64, (h + 1) * 64)
            o_ps = [aps.tile([P, 4, 128], F32, tag="ops", bufs=2,
                             name=f"o{c2}{h}{tp}{i}") for i in range(2)]
            pending = []

            def emit_attnv(s, es_bf):
                for tb in range(8):
                    # start=True zeroes the whole PSUM bank: only the FIRST
                    # chain of each o_ps tile may use it; the other three
                    # accumulate onto the bank zeroed by that first start.
                    nc.tensor.matmul(
                        o_ps[tb // 4][:, tb % 4, 0:65],
                        es_bf[:, tb * P:(tb + 1) * P],
                        V_sb[:, hh, s, :],
                        start=(s == 0 and tb % 4 == 0), stop=(s == NSC - 1),
                        skip_group_check=True)

            for s in range(NSC):
                if chase and s % SGT == 0:
                    yield s
                    for (ss, ee) in pending:
                        emit_attnv(ss, ee)
                    pending.clear()
                if inject and s in inject:
                    inject[s]()
                slab = aps.tile([P, TW], F32, tag="slab", bufs=2,
                                name=f"sl{c2}{h}{tp}{s}")
                # ISA limits matmul streams to 512 elements per AP dim
                for hf in range(2):
                    nc.tensor.matmul(
                        slab[:, hf * 512:(hf + 1) * 512],
                        KT_sb[c2][prow, s * P:(s + 1) * P],
                        QT_sb[c2][prow,
                                  tp * TW + hf * 512:tp * TW + (hf + 1) * 512],
                        start=True, stop=True, skip_group_check=True)
                eng = pattern[s % 8]
                if eng == 'A':
                    es_t = espool.tile([P, TW], BF16, tag="es", bufs=10)
                    nc.scalar.activation(es_t, slab, EXP,
                                         scale=SCALE / WK_SCALE,
                                         bias=ebias_sb[:, 0:1])
                    es_bf = es_t
                else:
                    es_t = espool.tile([P, TW], I16, tag="es", bufs=10)
                    nc.vector.tensor_scalar(es_t, slab, SCH_S0, SCH_S1,
                                            MULT, ADD)
                    es_bf = es_t.bitcast(BF16)
                pending.append((s, es_bf))
                # Software pipelining: keep attnV 2 tiles behind the exp
                # producers so the 8 blocked matmuls never clog the PE
                # wait queue (depth 4) ahead of the next scores matmul.
                if not chase and len(pending) > 2:
                    emit_attnv(*pending.pop(0))
            if chase:
                yield NSC
            for (ss, ee) in pending:
                emit_attnv(ss, ee)
            pending.clear()

            def evict():
                # normalize (per-partition recip) + PE transpose back.
                # custom-DVE recip straight off strided PSUM mis-addresses;
                # stage the 8 denominators into SBUF with a plain copy first.
                den8 = evpool.tile([P, 8], F32, tag="den8", bufs=2)
                for i in range(2):
                    nc.vector.tensor_copy(
                        den8[:, i * 4:(i + 1) * 4], o_ps[i][:, :, 64:65])
                rec8 = evpool.tile([P, 8], F32, tag="rec8", bufs=2)
                nc.vector.reciprocal_approx_fast(out=rec8, in_=den8)
                ocf = evpool.tile([P, 8, 64], BF16, tag="ocf", bufs=2)
                for tb in range(8):
                    nc.vector.tensor_scalar(
                        ocf[:, tb, :], o_ps[tb // 4][:, tb % 4, 0:64],
                        rec8[:, tb:tb + 1], None, MULT)
                tpp = aps.tile([P, 8, P], BF16, tag="slab", bufs=2,
                               name=f"tpp{c2}{h}{tp}")
                for tb in range(8):
                    nc.tensor.transpose(tpp[prow, tb, :], ocf[:, tb, :],
                                        ident_sb)
                nc.vector.tensor_copy(
                    OcatT[c2][prow, tcols],
                    tpp[prow, :, :].rearrange("p a b -> p (a b)"))

            yield ('EV', evict)

        def y_proj(tp, ms=(0, 1, 2, 3)):
            tcols = slice(tp * TW, (tp + 1) * TW)
            for m in ms:
                ps = aps.tile([P, TW], F32, tag="slab", bufs=2,
                              name=f"yps{tp}{m}")
                for hf in range(2):
                    for c2 in range(2):
                        nc.tensor.matmul(
                            ps[:, hf * 512:(hf + 1) * 512],
                            woT_sb[:, c2, m * P:(m + 1) * P],
                            OcatT[c2][:, tp * TW + hf * 512:
                                      tp * TW + (hf + 1) * 512],
                            start=(c2 == 0), stop=(c2 == 1),
                            skip_group_check=True)
                yo = ystg.tile([P, TW], F32, tag="yo")
                nc.vector.tensor_copy(yo, ps)
                nc.sync.dma_start(yT_r[:, m, tcols], yo)

        # ================= emission schedule =================
        ctx_dma(0)
        ctx_dma(1)
        # xT on the ACT queue so it transfers in parallel with ctx on sync
        nc.scalar.dma_start(xT_sb, xT.rearrange("(c p) t -> p c t", p=P))
        nc.sync.dma_start(wvT_sb, wvT.rearrange("(c p) m -> p c m", p=P))
        k_group(0)
        q_proj([(0, 0)])   # only the part the chase half needs

        u0 = unit_half(0, 0, 0, chase=True, pattern=PATTERN_CHASE)
        next(u0)   # yields at s=0 before emitting anything
        for g in range(NSG):
            if 2 <= g + 1 < NSG:
                ctx_dma(g + 1)
            v_group(g)
            if g + 1 < NSG:
                k_group(g + 1)
            if g == 0:
                q_proj([(1, 0)])
            elif g == 1:
                q_proj([(0, 1), (1, 1)])
            next(u0)   # scores/exp for segment g (+ attnV of segment g-1)
        prev_ev = None
        for item in u0:
            if isinstance(item, tuple):
                prev_ev = item[1]

        # Steady halves; each half's eviction is deferred into the NEXT
        # half (injected at s=2) so it overlaps that half's warmup.
        steady = [(0, 1, 0), (1, 0, 0), (1, 1, 0),
                  (0, 0, 1), (0, 1, 1), (1, 0, 1), (1, 1, 1)]
        for (c2, h, tp) in steady:
            inj = {2: prev_ev}
            if (c2, h, tp) == (0, 0, 1):
                inj[8] = lambda: y_proj(0, (0, 1))
                inj[20] = lambda: y_proj(0, (2, 3))
            for item in unit_half(c2, h, tp, inject=inj):
                if isinstance(item, tuple):
                    prev_ev = item[1]
        prev_ev()
        y_proj(1)

    persist.release()
    wts.release()


_NC_CACHE = None
LAST_RESULT = None


def _get_nc():
    global _NC_CACHE
    if _NC_CACHE is None:
        _NC_CACHE = _build_nc()
    return _NC_CACHE


def kernel(x, context, wq_w, wq_b, wk_w, wk_b, wv_w, wv_b, wo_w, wo_b):
    x = np.asarray(x)
    context = np.asarray(context)
    nc = _get_nc()

    ctx8 = [np.ascontiguousarray(context[b].T).astype(NPFP8) for b in range(B)]
    xT = [np.ascontiguousarray(x[b].T).astype(NPBF16) for b in range(B)]
    ident = np.eye(P, dtype=NPBF16)

    in_maps = []
    for c in range(N_CORES):
        b, g = c // 2, c % 2
        sl = slice(g * GE, (g + 1) * GE)
        in_maps.append({
            "ctx8": ctx8[b],
            "xT": xT[b],
            "wqT": np.ascontiguousarray(np.asarray(wq_w)[sl, :].T).astype(NPBF16),
            "wkT": np.ascontiguousarray(
                np.asarray(wk_w)[sl, :].T * WK_SCALE).astype(NPFP8),
            "wvT": np.ascontiguousarray(np.asarray(wv_w)[sl, :].T).astype(NPBF16),
            "woT": np.ascontiguousarray(np.asarray(wo_w)[:, sl].T).astype(NPBF16),
            "bq": np.ascontiguousarray(np.asarray(wq_b)[sl]).astype(np.float32),
            "bk": np.ascontiguousarray(
                np.asarray(wk_b)[sl] * WK_SCALE).astype(np.float32),
            "bv": np.ascontiguousarray(np.asarray(wv_b)[sl]).astype(np.float32),
            "ident": ident,
        })

    res = run_bass_kernel_spmd(nc, in_maps, core_ids=list(range(N_CORES)))
    global LAST_RESULT
    LAST_RESULT = res
    outs = res.results

    wo_b = np.asarray(wo_b, dtype=np.float32)
    y = np.empty((B, T, E), dtype=np.float32)
    for b in range(B):
        yt = outs[2 * b]["yT"] + outs[2 * b + 1]["yT"]
        y[b] = yt.T + wo_b
    return y


# revision 19
# speedup vs baseline: 1.1025x; 1.1025x over previous
"""Cross-attention Trainium2 kernel (nn_CrossAttention_8486855377137).

Sharding (8 cores): core c = (batch b = c//2, head-group g = c%2).
Each core handles one batch and 4 of the 8 heads (Q/K/V projections
column-sharded by head, wo row-sharded). Full softmax over S on device;
host sums the two partial wo outputs per batch and adds wo bias.

v4: three-engine exp + flipped attnV.
  - Head-major unit-halves: (c2-pair, head, t-width 1024); scores bf16
    64-contract with K stationary per s-tile, Q streamed 1024 cols.
  - exp split across engines: ACT exact exp; DVE/GPSIMD approximate exp
    via Schraudolph affine + f32->i16 round-convert bitcast to bf16
    (error cancels in softmax; sim: +1e-3 at 3/8 offload).
  - attnV flipped: es is the PE stationary (128x128 blocks), V (+ones
    denominator column) streams; out is [t, d|den] so the softmax
    normalize is a per-partition tensor_scalar, then PE transpose back.
  - K path fp8 DoubleRow (ctx fp8 + wk fp8 x512) as before.
"""

import numpy as np
import ml_dtypes

import concourse.bass as bass
import concourse.bacc as bacc
import concourse.tile as tile
import concourse.mybir as mybir
from concourse.bass_utils import run_bass_kernel_spmd

BF16 = mybir.dt.bfloat16
F32 = mybir.dt.float32
FP8 = mybir.dt.float8e4
I16 = mybir.dt.int16
EXP = mybir.ActivationFunctionType.Exp
ADD = mybir.AluOpType.add
MULT = mybir.AluOpType.mult
DR = mybir.MatmulPerfMode.DoubleRow
NPBF16 = ml_dtypes.bfloat16
NPFP8 = ml_dtypes.float8_e4m3

# Problem constants (hardcoded per contract)
B, T, S = 4, 2048, 4096
E, KV = 512, 2048
H, D = 8, 64
GE = 256            # head-group embed width (4 heads x 64)
SCALE = D ** -0.5   # 0.125
WK_SCALE = 512.0    # wk prescale (fp8 denormal avoidance); folded into exp scale
EXP_BIAS = -1.5     # exp(x + bias): cancels in softmax, shrinks es range

N_CORES = 8
P = 128
TW = 1024           # t-width per unit-half
NTP = T // TW       # 2 t-chunks
NSC = S // P        # 32 s-tiles
SGT = 4             # s-tiles per chase production step (512 cols)
NSG = NSC // SGT    # 8 production steps
KV_C = KV // P      # 16 contraction chunks for K/V proj
E_C = E // P        # 4 contraction chunks for Q proj

# Schraudolph constants (bf16: 7 mantissa bits)
SCH_K1 = 128.0 / np.log(2.0)
SCH_CORR = -7.4
SCH_S0 = SCH_K1 * SCALE / WK_SCALE
SCH_S1 = SCH_K1 * EXP_BIAS + 127.0 * 128.0 + SCH_CORR

# exp engine pattern per 8 s-tiles: A=ACT exact, D=DVE schraud.
# (GPSIMD cannot read PSUM, so it can't join the exp stream.)
PATTERN_STEADY = ('A', 'A', 'D', 'A', 'A', 'D', 'A', 'D')
PATTERN_CHASE = ('A',) * 8   # DVE busy with production evictions in chase


def _build_nc():
    nc = bacc.Bacc("TRN2", target_bir_lowering=False, debug=False)

    ctx8 = nc.dram_tensor("ctx8", [KV, S], FP8, kind="ExternalInput")
    xT = nc.dram_tensor("xT", [E, T], BF16, kind="ExternalInput")
    wqT = nc.dram_tensor("wqT", [E, GE], BF16, kind="ExternalInput")
    wkT = nc.dram_tensor("wkT", [KV, GE], FP8, kind="ExternalInput")
    wvT = nc.dram_tensor("wvT", [KV, GE], BF16, kind="ExternalInput")
    woT = nc.dram_tensor("woT", [GE, E], BF16, kind="ExternalInput")
    bq = nc.dram_tensor("bq", [GE], F32, kind="ExternalInput")
    bk = nc.dram_tensor("bk", [GE], F32, kind="ExternalInput")   # x512
    bv = nc.dram_tensor("bv", [GE], F32, kind="ExternalInput")
    ident = nc.dram_tensor("ident", [P, P], BF16, kind="ExternalInput")
    yT = nc.dram_tensor("yT", [E, T], F32, kind="ExternalOutput")

    with tile.TileContext(nc) as tc:
        _kernel_body(tc, nc, ctx8, xT, wqT, wkT, wvT, woT, bq, bk, bv,
                     ident, yT)
    nc.compile()
    return nc


def _kernel_body(tc, nc, ctx8, xT, wqT, wkT, wvT, woT, bq, bk, bv, ident, yT):
    wts = tc.alloc_tile_pool(name="wts", bufs=1)
    persist = tc.alloc_tile_pool(name="persist", bufs=1)

    # ---- DMA kickoff. Big streams on sync queue, weights on gpsimd queue ----
    ctx8_r = ctx8.rearrange("(c p) s -> p c s", p=P)
    yT_r = yT.rearrange("(m p) t -> p m t", p=P)

    wkT_sb = wts.tile([P, KV_C, GE], FP8, tag="wkT")
    nc.gpsimd.dma_start(wkT_sb, wkT.rearrange("(c p) m -> p c m", p=P))
    wqT_sb = wts.tile([P, E_C, GE], BF16, tag="wqT")
    nc.gpsimd.dma_start(wqT_sb, wqT.rearrange("(c p) m -> p c m", p=P))
    bq_sb = wts.tile([P, 2], F32, tag="bq")
    nc.gpsimd.dma_start(bq_sb, bq.rearrange("(c p) -> p c", p=P))
    bk_sb = wts.tile([P, 2], F32, tag="bk")
    nc.gpsimd.dma_start(bk_sb, bk.rearrange("(c p) -> p c", p=P))
    ident_sb = wts.tile([P, P], BF16, tag="ident")
    nc.gpsimd.dma_start(ident_sb, ident.ap())
    woT_sb = wts.tile([P, 2, E], BF16, tag="woT")
    nc.gpsimd.dma_start(woT_sb, woT.rearrange("(c p) m -> p c m", p=P))
    ebias_sb = wts.tile([P, 1], F32, tag="ebias")
    nc.vector.memset(ebias_sb, EXP_BIAS)
    # bv broadcast to all 128 partitions, used along free dim of V
    bv_bc = wts.tile([P, GE], F32, tag="bv_bc")
    bv_ap = bv.ap()
    bv_bcast_src = bass.AP(tensor=bv_ap.tensor, offset=bv_ap.offset,
                           ap=[[0, P]] + list(bv_ap.ap))
    nc.gpsimd.dma_start(out=bv_bc, in_=bv_bcast_src)

    xT_sb = wts.tile([P, E_C, T], BF16, tag="xT")
    wvT_sb = wts.tile([P, KV_C, GE], BF16, tag="wvT")

    # ---- persistent activation tiles ----
    QT_sb = [persist.tile([P, T], BF16, tag=f"QT{c}", name=f"QT{c}") for c in range(2)]
    KT_sb = [persist.tile([P, S], BF16, tag=f"KT{c}", name=f"KT{c}") for c in range(2)]
    # V bf16, head-major: [128, head(4), s-tile(32), 65]; col 64 = 1.0 (den)
    V_sb = persist.tile([P, 4, NSC, 65], BF16, tag="V", name="V")
    nc.vector.memset(V_sb[:, :, :, 64:65], 1.0)
    OcatT = [persist.tile([P, T], BF16, tag=f"Ocat{c}", name=f"Ocat{c}") for c in range(2)]

    with tc.tile_pool(name="aps", bufs=1, space="PSUM") as aps, \
         tc.tile_pool(name="c8pool", bufs=3) as c8pool, \
         tc.tile_pool(name="espool", bufs=10) as espool, \
         tc.tile_pool(name="evpool", bufs=2) as evpool, \
         tc.tile_pool(name="ystg", bufs=2) as ystg:

        ctx8_tiles = {}

        def ctx_dma(sg):
            """Fetch ctx half-group sg (512 cols) in fp8."""
            cols = slice(sg * 512, (sg + 1) * 512)
            t8 = c8pool.tile([P, KV_C, 512], FP8, tag="c8")
            nc.sync.dma_start(t8, ctx8_r[:, :, cols])
            ctx8_tiles[sg] = t8

        def k_group(sg):
            """KT[:, sg cols] for both c2 chunks; fp8 DoubleRow; x512."""
            ctx_t = ctx8_tiles[sg]
            for m in range(2):
                ps = aps.tile([P, 512], F32, tag="aux", bufs=2,
                              name=f"kps{sg}{m}")
                for cp in range(KV_C // 2):
                    nc.tensor.matmul(
                        ps,
                        wkT_sb[:, 2 * cp:2 * cp + 2, m * P:(m + 1) * P],
                        ctx_t[:, 2 * cp:2 * cp + 2, :],
                        start=(cp == 0), stop=(cp == KV_C // 2 - 1),
                        perf_mode=DR, skip_group_check=True)
                nc.vector.tensor_scalar_add(
                    KT_sb[m][:, sg * 512:(sg + 1) * 512],
                    ps, bk_sb[:, m:m + 1])

        def v_group(sg):
            """V s-tiles of half-group sg; bf16."""
            ctx_t = ctx8_tiles.pop(sg)
            for sp in range(2):
                ps = aps.tile([P, 512], F32, tag="aux", bufs=2,
                              name=f"vps{sg}{sp}")
                for st4 in (2 * sp, 2 * sp + 1):
                    for c in range(KV_C):
                        nc.tensor.matmul(
                            ps[:, (st4 % 2) * 256:(st4 % 2 + 1) * 256],
                            ctx_t[:, c, st4 * P:(st4 + 1) * P],
                            wvT_sb[:, c, :],
                            start=(c == 0), stop=(c == KV_C - 1),
                            skip_group_check=True)
                for st4 in (2 * sp, 2 * sp + 1):
                    idx = sg * SGT + st4
                    nc.vector.tensor_tensor(
                        V_sb[:, :, idx, 0:64],
                        ps[:, (st4 % 2) * 256:(st4 % 2 + 1) * 256].rearrange(
                            "p (h e) -> p h e", e=64),
                        bv_bc.rearrange("p (h e) -> p h e", e=64),
                        ADD)

        def q_proj(parts):
            for c2, tp in parts:
                for tn in range(2):
                    t = tp * 2 + tn
                    ps = aps.tile([P, 512], F32, tag="slab", bufs=2,
                                  name=f"qps{c2}{tp}{tn}")
                    for c in range(E_C):
                        nc.tensor.matmul(
                            ps,
                            wqT_sb[:, c, c2 * P:(c2 + 1) * P],
                            xT_sb[:, c, t * 512:(t + 1) * 512],
                            start=(c == 0), stop=(c == E_C - 1),
                            skip_group_check=True)
                    nc.vector.tensor_scalar_add(
                        QT_sb[c2][:, t * 512:(t + 1) * 512], ps,
                        bq_sb[:, c2:c2 + 1])

        def unit_half(c2, h, tp, chase=False, inject=None,
                      pattern=PATTERN_STEADY):
            """Generator: one head, 1024 t-cols, full softmax over S.

            scores: K-tile stationary (64-contract), Q streamed (1024).
            exp: engine per `pattern`. attnV: es stationary 128x128 blocks,
            V+ones streamed -> o_ps [t, 8tb, 65]; normalize per-partition;
            PE-transpose back into OcatT."""
            tcols = slice(tp * TW, (tp + 1) * TW)
            hh = 2 * c2 + h
            prow = slice(h * 64, (h + 1) * 64)
            o_ps = [aps.tile([P, 4, 128], F32, tag="ops", bufs=2,
                             name=f"o{c2}{h}{tp}{i}") for i in range(2)]
            pending = []

            def emit_attnv(s, es_bf):
                for tb in range(8):
                    # start=True zeroes the whole PSUM bank: only the FIRST
                    # chain of each o_ps tile may use it; the other three
                    # accumulate onto the bank zeroed by that first start.
                    nc.tensor.matmul(
                        o_ps[tb // 4][:, tb % 4, 0:65],
                        es_bf[:, tb * P:(tb + 1) * P],
                        V_sb[:, hh, s, :],
                        start=(s == 0 and tb % 4 == 0), stop=(s == NSC - 1),
                        skip_group_check=True)

            for s in range(NSC):
                if chase and s % SGT == 0:
                    yield s
                    for (ss, ee) in pending:
                        emit_attnv(ss, ee)
                    pending.clear()
                if inject and s in inject:
                    inject[s]()
                eng = pattern[s % 8]
                if eng == 'A':
                    # A-tiles recycle the "slab" psum slots through ACT only,
                    # keeping that chain at pure ACT cadence.
                    slab = aps.tile([P, TW], F32, tag="slab", bufs=2,
                                    name=f"sl{c2}{h}{tp}{s}")
                    # ISA limits matmul streams to 512 elements per AP dim
                    for hf in range(2):
                        nc.tensor.matmul(
                            slab[:, hf * 512:(hf + 1) * 512],
                            KT_sb[c2][prow, s * P:(s + 1) * P],
                            QT_sb[c2][prow, tp * TW + hf * 512:
                                      tp * TW + (hf + 1) * 512],
                            start=True, stop=True, skip_group_check=True)
                    es_t = espool.tile([P, TW], BF16, tag="es", bufs=10)
                    nc.scalar.activation(es_t, slab, EXP,
                                         scale=SCALE / WK_SCALE,
                                         bias=ebias_sb[:, 0:1])
                    es_bf = es_t
                else:
                    # D-tiles use their own "aux" psum slots (shared with the
                    # chase-only production) so the slow DVE recycle never
                    # blocks the ACT slab chain.
                    es_t = espool.tile([P, TW], I16, tag="es", bufs=10)
                    for hf in range(2):
                        dsl = aps.tile([P, 512], F32, tag="aux", bufs=2,
                                       name=f"dsl{c2}{h}{tp}{s}{hf}")
                        nc.tensor.matmul(
                            dsl,
                            KT_sb[c2][prow, s * P:(s + 1) * P],
                            QT_sb[c2][prow, tp * TW + hf * 512:
                                      tp * TW + (hf + 1) * 512],
                            start=True, stop=True, skip_group_check=True)
                        nc.vector.tensor_scalar(
                            es_t[:, hf * 512:(hf + 1) * 512], dsl,
                            SCH_S0, SCH_S1, MULT, ADD)
                    es_bf = es_t.bitcast(BF16)
                pending.append((s, es_bf))
                # Software pipelining: keep attnV 2 tiles behind the exp
                # producers so the 8 blocked matmuls never clog the PE
                # wait queue (depth 4) ahead of the next scores matmul.
                if not chase and len(pending) > 2:
                    emit_attnv(*pending.pop(0))
            if chase:
                yield NSC
            for (ss, ee) in pending:
                emit_attnv(ss, ee)
            pending.clear()

            def evict():
                # normalize (per-partition recip) + PE transpose back.
                # custom-DVE recip straight off strided PSUM mis-addresses;
                # stage the 8 denominators into SBUF with a plain copy first.
                den8 = evpool.tile([P, 8], F32, tag="den8", bufs=2)
                for i in range(2):
                    nc.vector.tensor_copy(
                        den8[:, i * 4:(i + 1) * 4], o_ps[i][:, :, 64:65])
                rec8 = evpool.tile([P, 8], F32, tag="rec8", bufs=2)
                nc.vector.reciprocal_approx_fast(out=rec8, in_=den8)
                ocf = evpool.tile([P, 8, 64], BF16, tag="ocf", bufs=2)
                for tb in range(8):
                    nc.vector.tensor_scalar(
                        ocf[:, tb, :], o_ps[tb // 4][:, tb % 4, 0:64],
                        rec8[:, tb:tb + 1], None, MULT)
                tpp = aps.tile([P, 8, P], BF16, tag="slab", bufs=2,
                               name=f"tpp{c2}{h}{tp}")
                for tb in range(8):
                    nc.tensor.transpose(tpp[prow, tb, :], ocf[:, tb, :],
                                        ident_sb)
                nc.vector.tensor_copy(
                    OcatT[c2][prow, tcols],
                    tpp[prow, :, :].rearrange("p a b -> p (a b)"))

            yield ('EV', evict)

        def y_proj(tp, ms=(0, 1, 2, 3)):
            tcols = slice(tp * TW, (tp + 1) * TW)
            for m in ms:
                ps = aps.tile([P, TW], F32, tag="slab", bufs=2,
                              name=f"yps{tp}{m}")
                for hf in range(2):
                    for c2 in range(2):
                        nc.tensor.matmul(
                            ps[:, hf * 512:(hf + 1) * 512],
                            woT_sb[:, c2, m * P:(m + 1) * P],
                            OcatT[c2][:, tp * TW + hf * 512:
                                      tp * TW + (hf + 1) * 512],
                            start=(c2 == 0), stop=(c2 == 1),
                            skip_group_check=True)
                yo = ystg.tile([P, TW], F32, tag="yo")
                nc.vector.tensor_copy(yo, ps)
                nc.sync.dma_start(yT_r[:, m, tcols], yo)

        # ================= emission schedule =================
        ctx_dma(0)
        ctx_dma(1)
        # xT on the ACT queue so it transfers in parallel with ctx on sync
        nc.scalar.dma_start(xT_sb, xT.rearrange("(c p) t -> p c t", p=P))
        nc.sync.dma_start(wvT_sb, wvT.rearrange("(c p) m -> p c m", p=P))
        k_group(0)
        q_proj([(0, 0)])   # only the part the chase half needs

        u0 = unit_half(0, 0, 0, chase=True, pattern=PATTERN_CHASE)
        next(u0)   # yields at s=0 before emitting anything
        for g in range(NSG):
            if 2 <= g + 1 < NSG:
                ctx_dma(g + 1)
            v_group(g)
            if g + 1 < NSG:
                k_group(g + 1)
            if g == 0:
                q_proj([(1, 0)])
            elif g == 1:
                q_proj([(0, 1), (1, 1)])
            next(u0)   # scores/exp for segment g (+ attnV of segment g-1)
        prev_ev = None
        for item in u0:
            if isinstance(item, tuple):
                prev_ev = item[1]

        # Steady halves; each half's eviction is deferred into the NEXT
        # half (injected at s=2) so it overlaps that half's warmup.
        steady = [(0, 1, 0), (1, 0, 0), (1, 1, 0),
                  (0, 0, 1), (0, 1, 1), (1, 0, 1), (1, 1, 1)]
        for (c2, h, tp) in steady:
            inj = {2: prev_ev}
            if (c2, h, tp) == (0, 0, 1):
                inj[8] = lambda: y_proj(0, (0, 1))
                inj[20] = lambda: y_proj(0, (2, 3))
            for item in unit_half(c2, h, tp, inject=inj):
                if isinstance(item, tuple):
                    prev_ev = item[1]
        prev_ev()
        y_proj(1)

    persist.release()
    wts.release()


_NC_CACHE = None
LAST_RESULT = None


def _get_nc():
    global _NC_CACHE
    if _NC_CACHE is None:
        _NC_CACHE = _build_nc()
    return _NC_CACHE


def kernel(x, context, wq_w, wq_b, wk_w, wk_b, wv_w, wv_b, wo_w, wo_b):
    x = np.asarray(x)
    context = np.asarray(context)
    nc = _get_nc()

    ctx8 = [np.ascontiguousarray(context[b].T).astype(NPFP8) for b in range(B)]
    xT = [np.ascontiguousarray(x[b].T).astype(NPBF16) for b in range(B)]
    ident = np.eye(P, dtype=NPBF16)

    in_maps = []
    for c in range(N_CORES):
        b, g = c // 2, c % 2
        sl = slice(g * GE, (g + 1) * GE)
        in_maps.append({
            "ctx8": ctx8[b],
            "xT": xT[b],
            "wqT": np.ascontiguousarray(np.asarray(wq_w)[sl, :].T).astype(NPBF16),
            "wkT": np.ascontiguousarray(
                np.asarray(wk_w)[sl, :].T * WK_SCALE).astype(NPFP8),
            "wvT": np.ascontiguousarray(np.asarray(wv_w)[sl, :].T).astype(NPBF16),
            "woT": np.ascontiguousarray(np.asarray(wo_w)[:, sl].T).astype(NPBF16),
            "bq": np.ascontiguousarray(np.asarray(wq_b)[sl]).astype(np.float32),
            "bk": np.ascontiguousarray(
                np.asarray(wk_b)[sl] * WK_SCALE).astype(np.float32),
            "bv": np.ascontiguousarray(np.asarray(wv_b)[sl]).astype(np.float32),
            "ident": ident,
        })

    res = run_bass_kernel_spmd(nc, in_maps, core_ids=list(range(N_CORES)))
    global LAST_RESULT
    LAST_RESULT = res
    outs = res.results

    wo_b = np.asarray(wo_b, dtype=np.float32)
    y = np.empty((B, T, E), dtype=np.float32)
    for b in range(B):
        yt = outs[2 * b]["yT"] + outs[2 * b + 1]["yT"]
        y[b] = yt.T + wo_b
    return y
